# revision 14
# baseline (speedup 1.0000x reference)
"""Multi-head self-attention (B=4, L=2048, E=768, H=12) on 8 trn2 NeuronCores.

Sharding: data-parallel over batch (4) x tensor-parallel over head halves (2).
Each core computes QKV projection for its 6 heads + causal attention, writing
y[b, :, g*384:(g+1)*384].

Device layout choices:
 - Host feeds x transposed (bf16) + a ones row; biases ride along as extra
   rows/columns so V needs no separate bias pass on PE, and Q/K biases fuse
   into the PSUM eviction on DVE.
 - QKV projection chains are emitted interleaved with the attention chunks
   that consume them: chunk c's Q/K chains and V k-tiles 4c..4c+3 are emitted
   right before attention on chunk c. This overlaps the PE-bound projection
   with the ACT-bound softmax and keeps the PE HAM clock warm.
 - Scores are computed transposed (S^T[k, q] = K @ Q^T per 128x512 block) so
   that softmax(P)@V contracts over k, which sits on partitions. Heads are
   processed in pairs: head 2t lives on partitions 0-63 and head 2t+1 on
   64-127 of the same Q^T/K^T tiles, so their Kc=64 score matmuls row-pack
   into disjoint PE row-groups and can run concurrently.
 - No max-subtraction in softmax: logits here are ~N(0,1) (the 1/8 scale is
   folded into the exp activation), so exp cannot overflow; masked entries
   are zeroed post-exp with affine_select (exact), which also covers regions
   the trimmed exp never wrote.
 - The softmax denominator comes from a ones-column appended to V (row 64 of
   the 65-row y_aug accumulator). A PE transpose turns y_aug row-major, where
   reciprocal+scale are cheap per-partition ops and the output DMA is
   row-contiguous.
"""

import numpy as np

B, L, E, H, D = 4, 2048, 768, 12, 64
NCORES = 8
HPC = H // 2          # heads per core = 6
DH = HPC * D          # 384 output cols per core
VW = HPC * (D + 1)    # 390: per-head 64 V cols + 1 ones col
EP = 896              # padded contraction rows: 768 + 1 bias row + pad (7*128)
KCP = EP // 128       # 7 contraction passes (V; Q/K use 6)
LT = L // 128         # 16 L-tiles
QC = L // 512         # 4 q-chunks
KG = 2                # k-tiles per exp batch (2 PSUM banks)

_compiled = None


def _emit(tc, nc, xT_d, wq_d, wk_d, wv_d, bqk_d, out_d):
    from contextlib import ExitStack

    import concourse.mybir as mybir
    from concourse.masks import make_identity

    f32 = mybir.dt.float32
    bf16 = mybir.dt.bfloat16
    Exp = mybir.ActivationFunctionType.Exp

    xT_v = xT_d.ap().rearrange("(n p) m -> n p m", p=128)
    wq_v = wq_d.ap().rearrange("(n p) m -> n p m", p=128)
    wk_v = wk_d.ap().rearrange("(n p) m -> n p m", p=128)
    wv_v = wv_d.ap().rearrange("(n p) m -> n p m", p=128)
    out_v = out_d.ap().rearrange("(n p) m -> n p m", p=128)

    with ExitStack() as ctx:
        const_pool = ctx.enter_context(tc.tile_pool(name="const", bufs=1))
        ident = const_pool.tile([128, 128], f32, name="ident")
        make_identity(nc, ident)
        bqk = const_pool.tile([128, 6], f32, name="bqk")
        nc.sync.dma_start(bqk[:], bqk_d.ap())

        in_pool = ctx.enter_context(tc.tile_pool(name="inp", bufs=1))
        xt = [in_pool.tile([128, L], bf16, name=f"xt{i}") for i in range(KCP)]
        wq = [in_pool.tile([128, DH], bf16, name=f"wq{i}") for i in range(6)]
        wk = [in_pool.tile([128, DH], bf16, name=f"wk{i}") for i in range(6)]
        wv = [in_pool.tile([128, VW], bf16, name=f"wv{i}") for i in range(KCP)]
        for i in range(KCP):
            nc.sync.dma_start(xt[i][:, 0:512], xT_v[i, :, 0:512])
        for m in range(3):  # m-chunk order so the first chains start early
            for i in range(6):
                nc.sync.dma_start(
                    wq[i][:, m * 128 : (m + 1) * 128],
                    wq_v[i, :, m * 128 : (m + 1) * 128],
                )
                nc.sync.dma_start(
                    wk[i][:, m * 128 : (m + 1) * 128],
                    wk_v[i, :, m * 128 : (m + 1) * 128],
                )
            if m == 0:
                for i in range(KCP):
                    nc.sync.dma_start(wv[i][:], wv_v[i, :, :])
        for i in range(KCP):
            nc.sync.dma_start(xt[i][:, 512:L], xT_v[i, :, 512:L])

        qkv_pool = ctx.enter_context(tc.tile_pool(name="qkv", bufs=1))
        QT = [qkv_pool.tile([128, L], bf16, name=f"qt{t}") for t in range(3)]
        KT = [qkv_pool.tile([128, L], bf16, name=f"kt{t}") for t in range(3)]
        VT = [qkv_pool.tile([128, VW], bf16, name=f"vt{t}") for t in range(LT)]

        # Static PSUM budget (8 banks): s_psum 2x2, ya 2x1, yt 1x1, proj 1x1.
        s_psum = ctx.enter_context(tc.tile_pool(name="s_psum", bufs=2, space="PSUM"))
        ya_psum = ctx.enter_context(tc.tile_pool(name="ya", bufs=2, space="PSUM"))
        yt_psum = ctx.enter_context(tc.tile_pool(name="yt", bufs=1, space="PSUM"))
        pj_psum = ctx.enter_context(tc.tile_pool(name="pj", bufs=1, space="PSUM"))

        # PE warmup during the input DMA wait: dense dummy matmuls lift the
        # HAM clock gate to 8/8 before the first real chain issues
        warm = const_pool.tile([128, 512], bf16, name="warm")
        nc.gpsimd.memset(warm[:], 0.0)
        wps = s_psum.tile([128, 1024], f32, tag="s", name="wps")
        for i in range(24):
            nc.tensor.matmul(
                wps[:, 0:512], warm[:, 0:128], warm[:], start=True, stop=True
            )


        p_pool = ctx.enter_context(tc.tile_pool(name="p_pool", bufs=18))
        ysb_pool = ctx.enter_context(tc.tile_pool(name="ysb", bufs=4))
        rc_pool = ctx.enter_context(tc.tile_pool(name="rc", bufs=8))
        stage_pool = ctx.enter_context(tc.tile_pool(name="stage", bufs=8))

        def emit_qk_chain(w_tiles, dst, bcol, m, c, pool=None):
            if pool is None:
                ps = pj_psum.tile([128, 512], f32, tag="pj", name="proj_ps")
            else:
                ps = pool.tile([128, 1024], f32, tag="s", name="proj_ps")
            for kc in range(6):
                nc.tensor.matmul(
                    ps[:, 0:512],
                    w_tiles[kc][:, m * 128 : (m + 1) * 128],
                    xt[kc][:, c * 512 : (c + 1) * 512],
                    start=(kc == 0),
                    stop=(kc == 5),
                )
            nc.vector.tensor_scalar_add(
                dst[m][:, c * 512 : (c + 1) * 512],
                ps[:, 0:512],
                bqk[:, bcol : bcol + 1],
            )

        def emit_v_chain(t, pool=None):
            if pool is None:
                ps = pj_psum.tile([128, 512], f32, tag="pj", name="projv_ps")
            else:
                ps = pool.tile([128, 1024], f32, tag="s", name="projv_ps")
            for kc in range(KCP):
                nc.tensor.matmul(
                    ps[:, 0:VW],
                    xt[kc][:, t * 128 : (t + 1) * 128],
                    wv[kc][:],
                    start=(kc == 0),
                    stop=(kc == KCP - 1),
                )
            nc.vector.tensor_copy(VT[t][:], ps[:, 0:VW])

        def chunk_chains(c, pools=(None,)):
            out = []
            for m in range(3):
                out.append((emit_qk_chain, (wq, QT, m, m, c)))
                out.append((emit_qk_chain, (wk, KT, 3 + m, m, c)))
            for t4 in range(4 * c, 4 * c + 4):
                out.append((emit_v_chain, (t4,)))
            return [
                (lambda fn=fn, args=args, pool=pools[i % len(pools)]: fn(
                    *args, pool=pool
                ))
                for i, (fn, args) in enumerate(out)
            ]

        for c in range(QC):
            nk = 4 * c + 4  # causal: k-tiles 0..nk-1

            if c == 0:  # chunk 0's chains up front (3-deep across pools);
                for ch in chunk_chains(0, pools=(None, s_psum, s_psum)):
                    ch()  # later chunks' chains are spread through the chunk
            pending_chains = chunk_chains(c + 1) if c + 1 < QC else []

            groups = [list(range(g, min(g + KG, nk))) for g in range(0, nk, KG)]
            stage = [
                stage_pool.tile([128, DH], f32, tag="st", name=f"stage_{c}_{s}")
                for s in range(4)
            ]
            def emit_scores(t):
                # One PSUM tile per k-tile holds both heads' S^T [128, 2x512];
                # the two Kc=64 matmuls alternate PE row-groups (partitions
                # 0-63 / 64-127) so they can run concurrently, and one exp /
                # affine_select covers both heads.
                p_tiles = []
                for ki in range(nk):
                    ps = s_psum.tile([128, 1024], f32, tag="s", name="s_ps")
                    for hp in range(2):
                        r = hp * 64
                        nc.tensor.matmul(
                            ps[:, hp * 512 : (hp + 1) * 512],
                            KT[t][r : r + 64, ki * 128 : (ki + 1) * 128],
                            QT[t][r : r + 64, c * 512 : (c + 1) * 512],
                            start=True,
                            stop=True,
                        )
                    pt = p_pool.tile([128, 1024], bf16, tag="p", name="p_t")
                    s0 = max(0, 128 * ki - 512 * c)  # q < s0 is all-invalid
                    if s0 == 0:
                        nc.scalar.activation(pt[:], ps[:], Exp, scale=0.125)
                    else:
                        pt3 = pt.rearrange("p (h w) -> p h w", h=2)
                        ps3 = ps.rearrange("p (h w) -> p h w", h=2)
                        nc.scalar.activation(
                            pt3[:, :, s0:512], ps3[:, :, s0:512], Exp, scale=0.125
                        )
                        nc.gpsimd.memset(pt3[:, :, 0:s0], 0.0)
                    if ki >= 4 * c:  # straddles the diagonal: zero k>q
                        pt3 = pt.rearrange("p (h w) -> p h w", h=2)
                        nc.gpsimd.affine_select(
                            pt3[:, :, s0:512],
                            pt3[:, :, s0:512],
                            compare_op=mybir.AluOpType.is_ge,
                            fill=0.0,
                            base=512 * c - 128 * ki + s0,
                            channel_multiplier=-1,
                            pattern=[[0, 2], [1, 512 - s0]],
                        )
                    p_tiles.append(pt)
                return p_tiles

            def emit_pv(t, p_tiles):
                # y_aug[65, 512] += V~[ki] (with ones col) contracted with P;
                # both heads' chains interleave so p tiles retire early
                ya = [
                    ya_psum.tile([65, 512], f32, tag="ya", name=f"ya{hp}")
                    for hp in range(2)
                ]
                for ki in range(nk):
                    for hp in range(2):
                        h = 2 * t + hp
                        nc.tensor.matmul(
                            ya[hp][:],
                            VT[ki][:, h * 65 : h * 65 + 65],
                            p_tiles[ki][:, hp * 512 : (hp + 1) * 512],
                            start=(ki == 0),
                            stop=(ki == nk - 1),
                        )
                for hp in range(2):
                    h = 2 * t + hp
                    # transpose 65x128 chunks back to row-major and normalize
                    ysb = ysb_pool.tile([65, 512], f32, tag="ysb", name="ysb")
                    nc.vector.tensor_copy(ysb[:], ya[hp][:])
                    yt = yt_psum.tile([128, 512], f32, tag="yt", name="yt")
                    for s in range(4):
                        nc.tensor.transpose(
                            yt[:, s * 128 : s * 128 + 65],
                            ysb[:, s * 128 : (s + 1) * 128],
                            ident[0:65, 0:65],
                        )
                    for s in range(4):
                        rc = rc_pool.tile([128, 1], f32, tag="rc", name="rc")
                        nc.vector.reciprocal(rc[:], yt[:, s * 128 + 64 : s * 128 + 65])
                        nc.vector.tensor_scalar_mul(
                            stage[s][:, h * 64 : (h + 1) * 64],
                            yt[:, s * 128 : s * 128 + 64],
                            rc[:],
                        )

            # software pipeline: next pair's scores overlap this pair's PV,
            # and next chunk's projection chains fill leftover PE slack
            pending = emit_scores(0)
            for t in range(3):
                nxt = emit_scores(t + 1) if t < 2 else None
                n_ch = (len(pending_chains) + 2 - t) // (3 - t)
                for _ in range(n_ch):
                    pending_chains.pop(0)()
                emit_pv(t, pending)
                pending = nxt
            assert not pending_chains
            for s in range(4):
                nc.sync.dma_start(out_v[c * 4 + s, :, :], stage[s][:])


def build():
    import concourse.mybir as mybir
    import concourse.tile as tile
    from concourse import bacc

    f32 = mybir.dt.float32
    bf16 = mybir.dt.bfloat16
    nc = bacc.Bacc("TRN2", target_bir_lowering=False, debug=False)
    xT_d = nc.dram_tensor("xT", [EP, L], bf16, kind="ExternalInput")
    wq_d = nc.dram_tensor("wq", [EP, DH], bf16, kind="ExternalInput")
    wk_d = nc.dram_tensor("wk", [EP, DH], bf16, kind="ExternalInput")
    wv_d = nc.dram_tensor("wv", [EP, VW], bf16, kind="ExternalInput")
    bqk_d = nc.dram_tensor("bqk", [128, 6], f32, kind="ExternalInput")
    out_d = nc.dram_tensor("out", [L, DH], f32, kind="ExternalOutput")

    with tile.TileContext(nc) as tc:
        _emit(tc, nc, xT_d, wq_d, wk_d, wv_d, bqk_d, out_d)
    nc.compile()
    return nc


def make_in_maps(x, Wq, Wk, Wv, bq, bk, bv):
    import ml_dtypes

    bf16 = ml_dtypes.bfloat16
    x = np.asarray(x, np.float32)
    Wq = np.asarray(Wq, np.float32)
    Wk = np.asarray(Wk, np.float32)
    Wv = np.asarray(Wv, np.float32)
    bq = np.asarray(bq, np.float32)
    bk = np.asarray(bk, np.float32)
    bv = np.asarray(bv, np.float32)
    in_maps = []
    for core in range(NCORES):
        b, g = divmod(core, 2)
        xc = np.zeros((EP, L), np.float32)
        xc[:E] = x[b].T
        xc[E] = 1.0
        wqc = np.zeros((EP, DH), np.float32)
        wqc[:E] = Wq[:, g * DH : (g + 1) * DH]
        wkc = np.zeros((EP, DH), np.float32)
        wkc[:E] = Wk[:, g * DH : (g + 1) * DH]
        # per-partition bias columns for the Q/K eviction: col m = q-proj
        # m-chunk, col 3+m = k-proj m-chunk
        bqkc = np.zeros((128, 6), np.float32)
        for m in range(3):
            bqkc[:, m] = bq[g * DH + m * 128 : g * DH + (m + 1) * 128]
            bqkc[:, 3 + m] = bk[g * DH + m * 128 : g * DH + (m + 1) * 128]
        wvc = np.zeros((EP, VW), np.float32)
        for hl in range(HPC):
            h = g * HPC + hl
            wvc[:E, hl * 65 : hl * 65 + 64] = Wv[:, h * 64 : (h + 1) * 64]
            wvc[E, hl * 65 : hl * 65 + 64] = bv[h * 64 : (h + 1) * 64]
            wvc[E, hl * 65 + 64] = 1.0
        in_maps.append(
            {
                "xT": xc.astype(bf16),
                "wq": wqc.astype(bf16),
                "wk": wkc.astype(bf16),
                "wv": wvc.astype(bf16),
                "bqk": bqkc,
            }
        )
    return in_maps


def kernel(x, Wq, Wk, Wv, bq, bk, bv, mask):
    global _compiled
    mask = np.asarray(mask)
    if not np.array_equal(mask != 0, np.tril(np.ones((L, L), bool))):
        raise ValueError("kernel is specialized for the causal (tril) mask")
    if _compiled is None:
        _compiled = build()
    nc = _compiled

    from concourse.bass_utils import run_bass_kernel_spmd

    in_maps = make_in_maps(x, Wq, Wk, Wv, bq, bk, bv)
    res = run_bass_kernel_spmd(nc, in_maps, core_ids=list(range(NCORES)))
    out = np.empty((B, L, E), np.float32)
    for core in range(NCORES):
        b, g = divmod(core, 2)
        out[b, :, g * DH : (g + 1) * DH] = res.results[core]["out"]
    return out


# revision 15
# speedup vs baseline: 1.0600x; 1.0600x over previous
"""Multi-head self-attention (B=4, L=2048, E=768, H=12) on 8 trn2 NeuronCores.

Sharding: data-parallel over batch (4) x tensor-parallel over head halves (2).
Each core computes QKV projection for its 6 heads + causal attention, writing
y[b, :, g*384:(g+1)*384].

Device layout choices:
 - Host feeds x transposed (bf16) + a ones row; biases ride along as extra
   rows/columns so V needs no separate bias pass on PE, and Q/K biases fuse
   into the PSUM eviction on DVE.
 - QKV projection chains are emitted interleaved with the attention chunks
   that consume them: chunk c's Q/K chains and V k-tiles 4c..4c+3 are emitted
   right before attention on chunk c. This overlaps the PE-bound projection
   with the ACT-bound softmax and keeps the PE HAM clock warm.
 - Scores are computed transposed (S^T[k, q] = K @ Q^T per 128x512 block) so
   that softmax(P)@V contracts over k, which sits on partitions. Heads are
   processed in pairs: head 2t lives on partitions 0-63 and head 2t+1 on
   64-127 of the same Q^T/K^T tiles, so their Kc=64 score matmuls row-pack
   into disjoint PE row-groups and can run concurrently.
 - No max-subtraction in softmax: logits here are ~N(0,1) (the 1/8 scale is
   folded into the exp activation), so exp cannot overflow; masked entries
   are zeroed post-exp with affine_select (exact), which also covers regions
   the trimmed exp never wrote.
 - The softmax denominator comes from a ones-column appended to V (row 64 of
   the 65-row y_aug accumulator). A PE transpose turns y_aug row-major, where
   reciprocal+scale are cheap per-partition ops and the output DMA is
   row-contiguous.
"""

import numpy as np

B, L, E, H, D = 4, 2048, 768, 12, 64
NCORES = 8
HPC = H // 2          # heads per core = 6
DH = HPC * D          # 384 output cols per core
VW = HPC * (D + 1)    # 390: per-head 64 V cols + 1 ones col
EP = 896              # padded contraction rows: 768 + 1 bias row + pad (7*128)
KCP = EP // 128       # 7 contraction passes (V; Q/K use 6)
LT = L // 128         # 16 L-tiles
QC = L // 512         # 4 q-chunks
KG = 2                # k-tiles per exp batch (2 PSUM banks)

_compiled = None


def _emit(tc, nc, xT_d, wq_d, wk_d, wv_d, bqk_d, out_d):
    from contextlib import ExitStack

    import concourse.mybir as mybir
    from concourse.masks import make_identity

    f32 = mybir.dt.float32
    bf16 = mybir.dt.bfloat16
    Exp = mybir.ActivationFunctionType.Exp

    xT_v = xT_d.ap().rearrange("(n p) m -> n p m", p=128)
    wq_v = wq_d.ap().rearrange("(n p) m -> n p m", p=128)
    wk_v = wk_d.ap().rearrange("(n p) m -> n p m", p=128)
    wv_v = wv_d.ap().rearrange("(n p) m -> n p m", p=128)
    out_v = out_d.ap().rearrange("(n p) m -> n p m", p=128)

    with ExitStack() as ctx:
        const_pool = ctx.enter_context(tc.tile_pool(name="const", bufs=1))
        ident = const_pool.tile([128, 128], f32, name="ident")
        make_identity(nc, ident)
        bqk = const_pool.tile([128, 6], f32, name="bqk")
        nc.sync.dma_start(bqk[:], bqk_d.ap())

        in_pool = ctx.enter_context(tc.tile_pool(name="inp", bufs=1))
        xt = [in_pool.tile([128, L], bf16, name=f"xt{i}") for i in range(KCP)]
        wq = [in_pool.tile([128, DH], bf16, name=f"wq{i}") for i in range(6)]
        wk = [in_pool.tile([128, DH], bf16, name=f"wk{i}") for i in range(6)]
        wv = [in_pool.tile([128, VW], bf16, name=f"wv{i}") for i in range(KCP)]
        for i in range(KCP):
            nc.sync.dma_start(xt[i][:, 0:512], xT_v[i, :, 0:512])
        for i in range(6):
            nc.sync.dma_start(wq[i][:], wq_v[i, :, :])
            nc.sync.dma_start(wk[i][:], wk_v[i, :, :])
        for i in range(KCP):
            nc.sync.dma_start(wv[i][:], wv_v[i, :, :])
        for i in range(KCP):
            nc.sync.dma_start(xt[i][:, 512:L], xT_v[i, :, 512:L])

        qkv_pool = ctx.enter_context(tc.tile_pool(name="qkv", bufs=1))
        QT = [qkv_pool.tile([128, L], bf16, name=f"qt{t}") for t in range(3)]
        KT = [qkv_pool.tile([128, L], bf16, name=f"kt{t}") for t in range(3)]
        VT = [qkv_pool.tile([128, VW], bf16, name=f"vt{t}") for t in range(LT)]

        # Static PSUM budget (8 banks): s_psum 2x2, ya 2x1, yt 1x1, proj 1x1.
        s_psum = ctx.enter_context(tc.tile_pool(name="s_psum", bufs=2, space="PSUM"))
        ya_psum = ctx.enter_context(tc.tile_pool(name="ya", bufs=2, space="PSUM"))
        yt_psum = ctx.enter_context(tc.tile_pool(name="yt", bufs=1, space="PSUM"))
        pj_psum = ctx.enter_context(tc.tile_pool(name="pj", bufs=1, space="PSUM"))

        # PE warmup during the input DMA wait: dense dummy matmuls lift the
        # HAM clock gate to 8/8 before the first real chain issues
        warm = const_pool.tile([128, 512], bf16, name="warm")
        nc.gpsimd.memset(warm[:], 0.0)
        wps = s_psum.tile([128, 1024], f32, tag="s", name="wps")
        for i in range(24):
            nc.tensor.matmul(
                wps[:, 0:512], warm[:, 0:128], warm[:], start=True, stop=True
            )


        p_pool = ctx.enter_context(tc.tile_pool(name="p_pool", bufs=18))
        ysb_pool = ctx.enter_context(tc.tile_pool(name="ysb", bufs=4))
        rc_pool = ctx.enter_context(tc.tile_pool(name="rc", bufs=8))
        stage_pool = ctx.enter_context(tc.tile_pool(name="stage", bufs=8))

        def emit_qk_chain(w_tiles, dst, bcol, m, c, pool=None):
            if pool is None:
                ps = pj_psum.tile([128, 512], f32, tag="pj", name="proj_ps")
            else:
                ps = pool.tile([128, 1024], f32, tag="s", name="proj_ps")
            for kc in range(6):
                nc.tensor.matmul(
                    ps[:, 0:512],
                    w_tiles[kc][:, m * 128 : (m + 1) * 128],
                    xt[kc][:, c * 512 : (c + 1) * 512],
                    start=(kc == 0),
                    stop=(kc == 5),
                )
            nc.vector.tensor_scalar_add(
                dst[m][:, c * 512 : (c + 1) * 512],
                ps[:, 0:512],
                bqk[:, bcol : bcol + 1],
            )

        def emit_v_chain(t, pool=None):
            if pool is None:
                ps = pj_psum.tile([128, 512], f32, tag="pj", name="projv_ps")
            else:
                ps = pool.tile([128, 1024], f32, tag="s", name="projv_ps")
            for kc in range(KCP):
                nc.tensor.matmul(
                    ps[:, 0:VW],
                    xt[kc][:, t * 128 : (t + 1) * 128],
                    wv[kc][:],
                    start=(kc == 0),
                    stop=(kc == KCP - 1),
                )
            nc.vector.tensor_copy(VT[t][:], ps[:, 0:VW])

        def chunk_chains(c, pools=(None,)):
            out = []
            for m in range(3):
                out.append((emit_qk_chain, (wq, QT, m, m, c)))
                out.append((emit_qk_chain, (wk, KT, 3 + m, m, c)))
            for t4 in range(4 * c, 4 * c + 4):
                out.append((emit_v_chain, (t4,)))
            return [
                (lambda fn=fn, args=args, pool=pools[i % len(pools)]: fn(
                    *args, pool=pool
                ))
                for i, (fn, args) in enumerate(out)
            ]

        for c in range(QC):
            nk = 4 * c + 4  # causal: k-tiles 0..nk-1

            if c == 0:  # chunk 0's chains up front (3-deep across pools);
                for ch in chunk_chains(0, pools=(None, s_psum, s_psum)):
                    ch()  # later chunks' chains are spread through the chunk
            pending_chains = chunk_chains(c + 1) if c + 1 < QC else []

            groups = [list(range(g, min(g + KG, nk))) for g in range(0, nk, KG)]
            stage = [
                stage_pool.tile([128, DH], f32, tag="st", name=f"stage_{c}_{s}")
                for s in range(4)
            ]
            def emit_scores(t):
                # One PSUM tile per k-tile holds both heads' S^T [128, 2x512];
                # the two Kc=64 matmuls alternate PE row-groups (partitions
                # 0-63 / 64-127) so they can run concurrently, and one exp /
                # affine_select covers both heads.
                p_tiles = []
                for ki in range(nk):
                    ps = s_psum.tile([128, 1024], f32, tag="s", name="s_ps")
                    for hp in range(2):
                        r = hp * 64
                        nc.tensor.matmul(
                            ps[:, hp * 512 : (hp + 1) * 512],
                            KT[t][r : r + 64, ki * 128 : (ki + 1) * 128],
                            QT[t][r : r + 64, c * 512 : (c + 1) * 512],
                            start=True,
                            stop=True,
                        )
                    pt = p_pool.tile([128, 1024], bf16, tag="p", name="p_t")
                    s0 = max(0, 128 * ki - 512 * c)  # q < s0 is all-invalid
                    if s0 == 0:
                        nc.scalar.activation(pt[:], ps[:], Exp, scale=0.125)
                    else:
                        pt3 = pt.rearrange("p (h w) -> p h w", h=2)
                        ps3 = ps.rearrange("p (h w) -> p h w", h=2)
                        nc.scalar.activation(
                            pt3[:, :, s0:512], ps3[:, :, s0:512], Exp, scale=0.125
                        )
                        nc.gpsimd.memset(pt3[:, :, 0:s0], 0.0)
                    if ki >= 4 * c:  # straddles the diagonal: zero k>q
                        pt3 = pt.rearrange("p (h w) -> p h w", h=2)
                        nc.gpsimd.affine_select(
                            pt3[:, :, s0:512],
                            pt3[:, :, s0:512],
                            compare_op=mybir.AluOpType.is_ge,
                            fill=0.0,
                            base=512 * c - 128 * ki + s0,
                            channel_multiplier=-1,
                            pattern=[[0, 2], [1, 512 - s0]],
                        )
                    p_tiles.append(pt)
                return p_tiles

            def emit_pv(t, p_tiles):
                # y_aug[65, 512] += V~[ki] (with ones col) contracted with P;
                # both heads' chains interleave so p tiles retire early
                ya = [
                    ya_psum.tile([65, 512], f32, tag="ya", name=f"ya{hp}")
                    for hp in range(2)
                ]
                for ki in range(nk):
                    for hp in range(2):
                        h = 2 * t + hp
                        nc.tensor.matmul(
                            ya[hp][:],
                            VT[ki][:, h * 65 : h * 65 + 65],
                            p_tiles[ki][:, hp * 512 : (hp + 1) * 512],
                            start=(ki == 0),
                            stop=(ki == nk - 1),
                        )
                for hp in range(2):
                    h = 2 * t + hp
                    # transpose 65x128 chunks back to row-major and normalize
                    ysb = ysb_pool.tile([65, 512], f32, tag="ysb", name="ysb")
                    nc.vector.tensor_copy(ysb[:], ya[hp][:])
                    yt = yt_psum.tile([128, 512], f32, tag="yt", name="yt")
                    for s in range(4):
                        nc.tensor.transpose(
                            yt[:, s * 128 : s * 128 + 65],
                            ysb[:, s * 128 : (s + 1) * 128],
                            ident[0:65, 0:65],
                        )
                    for s in range(4):
                        rc = rc_pool.tile([128, 1], f32, tag="rc", name="rc")
                        nc.vector.reciprocal(rc[:], yt[:, s * 128 + 64 : s * 128 + 65])
                        nc.vector.tensor_scalar_mul(
                            stage[s][:, h * 64 : (h + 1) * 64],
                            yt[:, s * 128 : s * 128 + 64],
                            rc[:],
                        )

            # software pipeline: next pair's scores overlap this pair's PV,
            # and next chunk's projection chains fill leftover PE slack
            pending = emit_scores(0)
            for t in range(3):
                nxt = emit_scores(t + 1) if t < 2 else None
                n_ch = (len(pending_chains) + 2 - t) // (3 - t)
                for _ in range(n_ch):
                    pending_chains.pop(0)()
                emit_pv(t, pending)
                pending = nxt
            assert not pending_chains
            for s in range(4):
                nc.sync.dma_start(out_v[c * 4 + s, :, :], stage[s][:])


def build():
    import concourse.mybir as mybir
    import concourse.tile as tile
    from concourse import bacc

    f32 = mybir.dt.float32
    bf16 = mybir.dt.bfloat16
    nc = bacc.Bacc("TRN2", target_bir_lowering=False, debug=False)
    xT_d = nc.dram_tensor("xT", [EP, L], bf16, kind="ExternalInput")
    wq_d = nc.dram_tensor("wq", [EP, DH], bf16, kind="ExternalInput")
    wk_d = nc.dram_tensor("wk", [EP, DH], bf16, kind="ExternalInput")
    wv_d = nc.dram_tensor("wv", [EP, VW], bf16, kind="ExternalInput")
    bqk_d = nc.dram_tensor("bqk", [128, 6], f32, kind="ExternalInput")
    out_d = nc.dram_tensor("out", [L, DH], f32, kind="ExternalOutput")

    with tile.TileContext(nc) as tc:
        _emit(tc, nc, xT_d, wq_d, wk_d, wv_d, bqk_d, out_d)
    nc.compile()
    return nc


def make_in_maps(x, Wq, Wk, Wv, bq, bk, bv):
    import ml_dtypes

    bf16 = ml_dtypes.bfloat16
    x = np.asarray(x, np.float32)
    Wq = np.asarray(Wq, np.float32)
    Wk = np.asarray(Wk, np.float32)
    Wv = np.asarray(Wv, np.float32)
    bq = np.asarray(bq, np.float32)
    bk = np.asarray(bk, np.float32)
    bv = np.asarray(bv, np.float32)
    in_maps = []
    for core in range(NCORES):
        b, g = divmod(core, 2)
        xc = np.zeros((EP, L), np.float32)
        xc[:E] = x[b].T
        xc[E] = 1.0
        wqc = np.zeros((EP, DH), np.float32)
        wqc[:E] = Wq[:, g * DH : (g + 1) * DH]
        wkc = np.zeros((EP, DH), np.float32)
        wkc[:E] = Wk[:, g * DH : (g + 1) * DH]
        # per-partition bias columns for the Q/K eviction: col m = q-proj
        # m-chunk, col 3+m = k-proj m-chunk
        bqkc = np.zeros((128, 6), np.float32)
        for m in range(3):
            bqkc[:, m] = bq[g * DH + m * 128 : g * DH + (m + 1) * 128]
            bqkc[:, 3 + m] = bk[g * DH + m * 128 : g * DH + (m + 1) * 128]
        wvc = np.zeros((EP, VW), np.float32)
        for hl in range(HPC):
            h = g * HPC + hl
            wvc[:E, hl * 65 : hl * 65 + 64] = Wv[:, h * 64 : (h + 1) * 64]
            wvc[E, hl * 65 : hl * 65 + 64] = bv[h * 64 : (h + 1) * 64]
            wvc[E, hl * 65 + 64] = 1.0
        in_maps.append(
            {
                "xT": xc.astype(bf16),
                "wq": wqc.astype(bf16),
                "wk": wkc.astype(bf16),
                "wv": wvc.astype(bf16),
                "bqk": bqkc,
            }
        )
    return in_maps


def kernel(x, Wq, Wk, Wv, bq, bk, bv, mask):
    global _compiled
    mask = np.asarray(mask)
    if not np.array_equal(mask != 0, np.tril(np.ones((L, L), bool))):
        raise ValueError("kernel is specialized for the causal (tril) mask")
    if _compiled is None:
        _compiled = build()
    nc = _compiled

    from concourse.bass_utils import run_bass_kernel_spmd

    in_maps = make_in_maps(x, Wq, Wk, Wv, bq, bk, bv)
    res = run_bass_kernel_spmd(nc, in_maps, core_ids=list(range(NCORES)))
    out = np.empty((B, L, E), np.float32)
    for core in range(NCORES):
        b, g = divmod(core, 2)
        out[b, :, g * DH : (g + 1) * DH] = res.results[core]["out"]
    return out
